# revision 1
# baseline (speedup 1.0000x reference)
"""Trainium2 Bass kernel for EquivariantPPFAttention (gnn_message_passing).

Contract: kernel(**inputs) takes FULL unsharded inputs (as produced by
reference.setup_inputs()) and returns the FULL [N, OUT, 3] float32 output.

Strategy (data-parallel over query points N across 8 NeuronCores):
  - shard q_pts / neighbor_indices across cores; replicate everything else.
  - one combined gather table comb[M, 512B]: s_feats row in bf16 (384B) +
    s_pts/normals in f32 (24B) + pad. Per query-tile of 128, dma_gather
    pulls all 128*32 neighbor rows (4 gathers of 1024 idxs - HW limit).
  - K-sum of the bf16 s_feats part on DVE (strided reduce, f32 accum);
    pts/normals extracted to a packed buffer for the PPF phase.
  - PPF angles via DVE arithmetic + ACT Sqrt/Arctan
    (atan2(r,y) = atan(r/y) + pi*[y<0] for r>=0); the four PPF features
    are written interleaved into one p4[:, t, ci, k] tensor.
  - tiny MLP on TensorE with rows on the free dim; two query-tiles packed
    per matmul via block-diagonal weights; mean-over-K folded into W3; the
    1/pi PPF normalization folded into W1; 1/K of the value path folded
    into Wv.
  - MLP input pack: per tile one PE transpose of p4 ([128 q, (ci k)] ->
    [(ci k), 128 q]) then ONE 4-row DMA (dst [4,4096] row-major ==
    src [128,128] partition-major traversal), alternating SP/ACT/Pool
    issue queues so no single sequencer serializes the pack; pf columns
    are k-major (col=k*128+q), so the K-mean reduce uses a strided view.
    h1/h2 run in f16 (weights cast on host); after the K-sum all f32.
    PSUM->SBUF staging copies ride the ACT engine to keep DVE free.
  - output DMAs issue from the Pool queue to keep the SP queue free for
    pack DMAs (in-order sequencers otherwise stall the next pair's pack).
"""

import math
import os
import numpy as np

N = 20000
M = 20000
K = 32
D = 64
HID = 64
OUT = 192
PPF_OUT = 64
N_CORES = 8
PI = math.pi

ES = 128          # f32 elems per comb row (512 B)
SFW = 96          # f32 slots holding the 192 bf16 s_feats values
PNO = 96          # f32 slot offset of pts/normals (6 floats)
NI = 1024         # idxs per dma_gather (HW-stable limit)
GPT = (128 * K) // NI   # gathers per query tile (4)
KPG = K // GPT    # k-blocks per gather (8)

_NC_CACHE = {}


def _build_nc(T, stage="full", loop=None):
    """Per-core Bass program for T query-tiles of 128.

    stage: debug bisection point - gather | ppf | mlp | full.
    loop: if set, repeat the whole body N times (for timing).
    """
    from contextlib import ExitStack, nullcontext
    from concourse import bacc, bass, mybir, tile

    assert T % 2 == 0
    NPAIR = T // 2
    NQ = 128 * T
    f32 = mybir.dt.float32
    bf16 = mybir.dt.bfloat16
    f16 = mybir.dt.float16
    i16 = mybir.dt.int16
    AF = mybir.ActivationFunctionType
    ALU = mybir.AluOpType

    nc = bacc.Bacc("TRN2", target_bir_lowering=False, debug=False)

    comb_in = nc.dram_tensor("comb", [M, ES], f32, kind="ExternalInput")
    qp_in = nc.dram_tensor("qp", [128, T, 3], f16, kind="ExternalInput")
    idx_in = nc.dram_tensor("idx16", [128, T, GPT, NI // 16], i16,
                            kind="ExternalInput")
    w1b_in = nc.dram_tensor("w1b", [8, 128], f16, kind="ExternalInput")
    b1b_in = nc.dram_tensor("b1b", [128, 1], f32, kind="ExternalInput")
    w2b_in = nc.dram_tensor("w2b", [128, 128], f16, kind="ExternalInput")
    b2b_in = nc.dram_tensor("b2b", [128, 1], f32, kind="ExternalInput")
    w3b_in = nc.dram_tensor("w3b", [128, 128], f32, kind="ExternalInput")
    b3b_in = nc.dram_tensor("b3b", [128, 1], f32, kind="ExternalInput")
    wgb_in = nc.dram_tensor("wgb", [128, 3, 128], f32, kind="ExternalInput")
    bgb_in = nc.dram_tensor("bgb", [128, 3], f32, kind="ExternalInput")
    wvb_in = nc.dram_tensor("wvb", [128, 3, 128], f32, kind="ExternalInput")
    ident_in = nc.dram_tensor("ident", [128, 128], f32, kind="ExternalInput")

    SIMPLEOUT = bool(int(os.environ.get("BENCH_SIMPLEOUT", "0")))
    if stage in ("full", "mlponly"):
        if SIMPLEOUT:
            out_dev = nc.dram_tensor(
                "out", [128, 9 * NQ], f32, kind="ExternalOutput"
            )
        else:
            out_dev = nc.dram_tensor(
                "out", [3, OUT, NQ], f32, kind="ExternalOutput"
            )
        dbg = None
    else:
        DBGW = {
            "gather": T * K * 8 + T * 192,
            "ppf": 4 * T * K,
            "mlp": 128 * K + 128 + 128 + 3 * 128,
            "gonly": T * 8,
            "gred": T * 8,
            "dbgdma": T * K * 8 + T * 192,
        }[stage]
        out_dev = None
        dbg = nc.dram_tensor("dbg", [128, DBGW], f32, kind="ExternalOutput")

    with tile.TileContext(nc) as tc, ExitStack() as ctx:
        const = ctx.enter_context(tc.tile_pool(name="const", bufs=1))
        gpool = ctx.enter_context(tc.tile_pool(name="gpool", bufs=2))
        gath = ctx.enter_context(tc.tile_pool(name="gath", bufs=1))
        planes = ctx.enter_context(tc.tile_pool(name="planes", bufs=1))
        temps = ctx.enter_context(tc.tile_pool(name="temps", bufs=2))
        mlpp = ctx.enter_context(tc.tile_pool(name="mlpp", bufs=1))
        small = ctx.enter_context(tc.tile_pool(name="small", bufs=2))
        psmlp = ctx.enter_context(tc.tile_pool(name="psmlp", bufs=3, space="PSUM"))
        pssm = ctx.enter_context(tc.tile_pool(name="pssm", bufs=3, space="PSUM"))
        pstp = ctx.enter_context(tc.tile_pool(name="pstp", bufs=2, space="PSUM"))

        def cload(name, dram, shape, dt=f32):
            t = const.tile(shape, dt, tag=name, name=name)
            if len(shape) > 3:
                dims = " ".join(f"d{i}" for i in range(len(shape) - 1))
                pat = f"p {dims} -> p ({dims})"
                nc.sync.dma_start(t[:].rearrange(pat), dram.ap().rearrange(pat))
            else:
                nc.sync.dma_start(t[:], dram.ap())
            return t

        qp_t = cload("qp", qp_in, [128, T, 3], f16)
        idx_t = cload("idx16", idx_in, [128, T, GPT, NI // 16], i16)
        w1b_t = cload("w1b", w1b_in, [8, 128], f16)
        b1b_t = cload("b1b", b1b_in, [128, 1])
        w2b_t = cload("w2b", w2b_in, [128, 128], f16)
        b2b_t = cload("b2b", b2b_in, [128, 1])
        w3b_t = cload("w3b", w3b_in, [128, 128])
        b3b_t = cload("b3b", b3b_in, [128, 1])
        wgb_t = cload("wgb", wgb_in, [128, 3, 128])
        bgb_t = cload("bgb", bgb_in, [128, 3])
        wvb_t = cload("wvb", wvb_in, [128, 3, 128])
        ident_t = cload("ident", ident_in, [128, 128])

        _loop_ctx = tc.For_i(0, loop, 1) if loop else nullcontext()
        with _loop_ctx:
            # ---- gather + per-tile K-reduce + pn extraction ----
            nbbuf = gath.tile([128, T, K, 8], f16, tag="nbbuf")
            sfsum = gath.tile([128, T, 192], f32, tag="sfsum")

            do_gather = stage not in ("dbgdma", "mlponly")
            do_reduce = stage not in ("gonly", "dbgdma", "mlponly")

            if stage in ("dbgdma", "mlponly"):
                nc.vector.memset(nbbuf[:].rearrange("p t k c -> p (t k c)"), 0.25)
                nc.vector.memset(sfsum[:].rearrange("p t c -> p (t c)"), 0.25)

            NORED = bool(int(os.environ.get("BENCH_NORED", "0")))
            if NORED and stage == "full":
                nc.vector.memset(nbbuf[:].rearrange("p t k c -> p (t k c)"), 0.25)
                nc.vector.memset(sfsum[:].rearrange("p t c -> p (t c)"), 0.25)

            gt_last = None
            if do_gather:
                for t in range(T):
                    gt = gpool.tile([128, K, ES], f32, tag="gt", name="gt")
                    gt_last = gt
                    for g in range(GPT):
                        nc.gpsimd.dma_gather(
                            out_ap=gt[:, g * KPG : (g + 1) * KPG, :],
                            in_ap=comb_in.ap(),
                            idxs_ap=idx_t[:, t, g, :],
                            num_idxs=NI,
                            num_idxs_reg=NI,
                            elem_size=ES,
                        )
                    if do_reduce and not NORED:
                        # K-sum of the bf16 s_feats block via a contiguous
                        # tree-add (f16 intermediates, f32 final level)
                        gtb = gt[:].bitcast(bf16)          # [128, K, 256]
                        ga = gpool.tile([128, 16, 256], f16, tag="ga", bufs=2)
                        nc.vector.tensor_tensor(
                            ga[:, :, 0:192], gtb[:, 0:16, 0:192],
                            gtb[:, 16:32, 0:192], ALU.add,
                        )
                        for lv in (8, 4, 2):
                            nc.vector.tensor_tensor(
                                ga[:, 0:lv, 0:192], ga[:, 0:lv, 0:192],
                                ga[:, lv : 2 * lv, 0:192], ALU.add,
                            )
                        nc.vector.tensor_tensor(
                            sfsum[:, t, :], ga[:, 0, 0:192], ga[:, 1, 0:192],
                            ALU.add,
                        )
                        # pts/normals (f32) -> nbbuf[:, t, :, 0:6]
                        nc.scalar.activation(
                            nbbuf[:, t, :, 0:6], gt[:, :, PNO : PNO + 6], AF.Copy
                        )
            if stage in ("gonly", "gred"):
                sb = gath.tile([128, T * 8], f32, tag="sdbg")
                nc.vector.tensor_copy(
                    sb[:],
                    nbbuf[:, 0, :, :].rearrange("p k c -> p (k c)")[:, : T * 8],
                )
                nc.sync.dma_start(dbg.ap()[:, : T * 8], sb[:])
            elif stage in ("gather", "dbgdma"):
                nc.sync.dma_start(
                    dbg.ap()[:, : T * K * 8],
                    nbbuf[:].rearrange("p t k c -> p (t k c)"),
                )
                nc.sync.dma_start(
                    dbg.ap()[:, T * K * 8 :],
                    sfsum[:].rearrange("p t c -> p (t c)"),
                )
            else:
                # ---- PPF + MLP, emitted in tile chunks so gather/PPF of
                # chunk c+1 overlaps the MLP pipeline of chunk c ----
                FW = T * K
                TT = nc.vector.tensor_tensor
                STT = nc.vector.scalar_tensor_tensor

                # interleaved PPF planes: p4[:, t, ci, k] (ci-major per tile)
                p4 = planes.tile([128, T, 4, K], f32, tag="p4", name="p4")

                def plane_view(ci):
                    # [128, FW] view of feature ci (strided; debug stages only)
                    return p4[:, :, ci, :].rearrange("p t k -> p (t k)")

                RW = 128 * K  # rows per query-tile (4096)
                HC = RW // 2
                NOPACK = bool(int(os.environ.get("BENCH_NOPACK", "0")))
                NOVAL = bool(int(os.environ.get("BENCH_NOVAL", "0")))
                NOGATE = bool(int(os.environ.get("BENCH_NOGATE", "0")))
                NOMLP12 = bool(int(os.environ.get("BENCH_NOMLP12", "0")))
                V1PACK = bool(int(os.environ.get("BENCH_V1PACK", "0")))
                PACKNODMA = bool(int(os.environ.get("BENCH_PACKNODMA", "0")))
                PACKNOTP = bool(int(os.environ.get("BENCH_PACKNOTP", "0")))

                if stage in ("full", "mlponly") and not SIMPLEOUT:
                    out_re = out_dev.ap().rearrange(
                        "c (jj p) q -> p c jj q", jj=3
                    )

                def emit_ppf(t0, tn):
                    with nc.allow_low_precision(reason="f16 PPF chain"):
                        _emit_ppf(t0, tn)

                def _emit_ppf(t0, tn):
                    nb = nbbuf[:, t0 : t0 + tn]

                    def ttile(tag):
                        return temps.tile([128, tn * K], f16, tag=tag, name=tag)

                    def v3(t_):
                        return t_[:].rearrange("p (t k) -> p t k", k=K)

                    npv = [nb[:, :, :, c] for c in range(3)]
                    nnv = [nb[:, :, :, 3 + c] for c in range(3)]
                    qnv = [
                        nb[:, :, 0, 3 + c].to_broadcast([128, tn, K])
                        for c in range(3)
                    ]
                    qpv = [
                        qp_t[:, t0 : t0 + tn, c].to_broadcast([128, tn, K])
                        for c in range(3)
                    ]

                    vd = []
                    for c in range(3):
                        t_ = ttile(f"vd{c}")
                        TT(v3(t_), npv[c], qpv[c], ALU.subtract)
                        vd.append(t_)

                    def dot_views(av, bv, out_tag):
                        m0 = ttile("dm0")
                        TT(m0[:], av[0], bv[0], ALU.mult)
                        m1 = ttile("dm1")
                        TT(m1[:], av[1], bv[1], ALU.mult)
                        s = ttile(out_tag)
                        TT(s[:], m0[:], m1[:], ALU.add)
                        m2 = ttile("dm0")
                        TT(m2[:], av[2], bv[2], ALU.mult)
                        TT(s[:], s[:], m2[:], ALU.add)
                        return s

                    def cross_views(av, bv):
                        outs = []
                        for c in range(3):
                            i, j = (c + 1) % 3, (c + 2) % 3
                            m0 = ttile("cm0")
                            TT(m0[:], av[i], bv[j], ALU.mult)
                            m1 = ttile("cm1")
                            TT(m1[:], av[j], bv[i], ALU.mult)
                            o = ttile(f"cr{c}")
                            TT(o[:], m0[:], m1[:], ALU.subtract)
                            outs.append(o)
                        return outs

                    vdv = [v3(t_) for t_ in vd]

                    # pass A: all dots/crosses (DVE); pass B: the four
                    # Sqrts back-to-back then the three Arctans, so ACT pays
                    # two activation-table swaps per body instead of six
                    # (Sqrt shares no table set with Arctan/Sigmoid/Relu).
                    dd = dot_views(vdv, vdv, "y_")

                    pairs = [(qnv, vdv), (nnv, vdv), (qnv, nnv)]
                    ys, rss = [], []
                    for ci, (av, bv) in enumerate(pairs, start=1):
                        y = dot_views(av, bv, f"y{ci}")
                        cr = cross_views(av, bv)
                        crv = [c_[:] for c_ in cr]
                        rs = dot_views(crv, crv, f"rs{ci}")
                        ys.append(y)
                        rss.append(rs)

                    nc.scalar.activation(
                        p4[:, t0 : t0 + tn, 0, :], v3(dd), AF.Sqrt
                    )
                    rs_r = []
                    for ci in range(3):
                        r = ttile(f"r{ci}")
                        nc.scalar.activation(r[:], rss[ci][:], AF.Sqrt)
                        rs_r.append(r)

                    for ci in range(3):
                        y = ys[ci]
                        iy = ttile("cm1")
                        nc.vector.reciprocal(iy[:], y[:])
                        tq = ttile("dm0")
                        TT(tq[:], rs_r[ci][:], iy[:], ALU.mult)
                        at = ttile("dm1")
                        nc.scalar.activation(at[:], tq[:], AF.Arctan)
                        ind = ttile("cr1")
                        nc.vector.tensor_scalar(ind[:], y[:], 0.0, None, ALU.is_lt)
                        STT(
                            p4[:, t0 : t0 + tn, ci + 1, :],
                            v3(ind), PI, v3(at), ALU.mult, ALU.add,
                        )

                def emit_pair(j):
                    pf = mlpp.tile([8, RW], f16, tag="pf", bufs=2)
                    if NOPACK:
                        nc.vector.memset(pf[:], 0.25)
                    elif V1PACK:
                        for t2 in range(2):
                            t_abs = 2 * j + t2
                            for ci in range(4):
                                nc.sync.dma_start(
                                    pf[t2 * 4 + ci : t2 * 4 + ci + 1, :],
                                    p4[:, t_abs, ci, :],
                                )
                    else:
                        # pack via PE transpose: p4 tile slab [128 q, (ci k)]
                        # -> [(ci k), 128 q]; then 4 row-DMAs per tile with
                        # 512B descriptors, spread over 3 DMA-issue engines.
                        pts_s = mlpp.tile([128, 2, 128], f16, tag="pts", bufs=3)
                        if PACKNOTP:
                            nc.vector.memset(
                                pts_s[:].rearrange("p t q -> p (t q)"), 0.25
                            )
                        else:
                            for t2 in range(2):
                                t_abs = 2 * j + t2
                                tp = pstp.tile([128, 128], f32, tag="pstp")
                                nc.tensor.transpose(
                                    tp[:],
                                    p4[:, t_abs, :, :].rearrange(
                                        "p c k -> p (c k)"
                                    ),
                                    ident_t[:],
                                )
                                nc.scalar.activation(
                                    pts_s[:, t2, :], tp[:], AF.Copy
                                )
                        if PACKNODMA:
                            nc.vector.memset(pf[:], 0.25)
                        else:
                            # one DMA per tile: dst [4, 4096] row-major ==
                            # src [128, 128] partition-major traversal
                            # (dst col k*128+q <- src partition ci*32+k col q)
                            engs = [nc.sync, nc.scalar, nc.gpsimd]
                            for t2 in range(2):
                                eng = engs[(j * 2 + t2) % 3]
                                eng.dma_start(
                                    pf[t2 * 4 : (t2 + 1) * 4, :],
                                    pts_s[:, t2, :],
                                )

                    # value-path transposes depend only on sfsum: run them
                    # on PE/ACT while the h1/h2 stream occupies the pipeline
                    if stage != "mlp" and not NOVAL:
                        av_e = sfsum[:, 2 * j : 2 * j + 2, :].rearrange(
                            "p t (d c) -> p c (t d)", c=3
                        )
                        aggs3 = small.tile([128, 3, 128], f32, tag="aggs3")
                        for c in range(3):
                            tpv = pssm.tile([128, 128], f32, tag="pssm")
                            nc.tensor.transpose(tpv[:], av_e[:, c, :], ident_t[:])
                            nc.scalar.activation(aggs3[:, c, :], tpv[:], AF.Copy)

                    ksum = small.tile([128, 128], f32, tag="ksum")
                    if NOMLP12:
                        nc.vector.memset(ksum[:], 0.25)
                    kparts = []
                    for hh in range(0 if NOMLP12 else 2):
                        h1s = mlpp.tile([128, HC], f16, tag="h1s", bufs=2)
                        for ch in range(HC // 512):
                            sl = slice(ch * 512, (ch + 1) * 512)
                            slg = slice(
                                hh * HC + ch * 512, hh * HC + (ch + 1) * 512
                            )
                            h1p = psmlp.tile([128, 512], f32, tag="psmlp")
                            nc.tensor.matmul(
                                h1p[:], w1b_t[:], pf[:, slg], start=True, stop=True
                            )
                            nc.scalar.activation(
                                h1s[:, sl], h1p[:], AF.Relu, bias=b1b_t[:]
                            )
                        h2s = mlpp.tile([128, HC], f16, tag="h2s", bufs=2)
                        for ch in range(HC // 512):
                            sl = slice(ch * 512, (ch + 1) * 512)
                            h2p = psmlp.tile([128, 512], f32, tag="psmlp")
                            nc.tensor.matmul(
                                h2p[:], w2b_t[:], h1s[:, sl], start=True, stop=True
                            )
                            if ch % 2 == 0:
                                nc.vector.tensor_scalar(
                                    h2s[:, sl], h2p[:], b2b_t[:], 0.0,
                                    ALU.add, ALU.max,
                                )
                            else:
                                nc.scalar.activation(
                                    h2s[:, sl], h2p[:], AF.Relu, bias=b2b_t[:]
                                )
                        # cols are k-major (col = k*128 + q): each hh holds
                        # k-block hh*16..hh*16+15 for all 128 queries
                        kp = small.tile([128, 128], f32, tag=f"kp{hh}")
                        nc.vector.reduce_sum(
                            kp[:],
                            h2s[:].rearrange("p (k q) -> p q k", q=128),
                            mybir.AxisListType.X,
                        )
                        kparts.append(kp)
                    if not NOMLP12:
                        TT(ksum[:], kparts[0][:], kparts[1][:], ALU.add)

                    pmp = pssm.tile([128, 128], f32, tag="pssm")
                    nc.tensor.matmul(pmp[:], w3b_t[:], ksum[:], start=True, stop=True)
                    pms = small.tile([128, 128], f32, tag="pms")
                    nc.vector.tensor_scalar_add(pms[:], pmp[:], b3b_t[:])

                    if stage == "mlp" and j == 0:
                        nc.sync.dma_start(dbg.ap()[:, : RW // 2], h2s[:])
                        nc.sync.dma_start(dbg.ap()[:, RW : RW + 128], ksum[:])
                        nc.sync.dma_start(dbg.ap()[:, RW + 128 : RW + 256], pms[:])

                    gates = []
                    for jj in range(3):
                        gs = small.tile(
                            [128, 128], f32, tag=f"gate{jj}", name=f"gate{jj}"
                        )
                        if NOGATE:
                            nc.vector.memset(gs[:], 0.5)
                            gates.append(gs)
                            continue
                        gp = pssm.tile([128, 128], f32, tag="pssm")
                        nc.tensor.matmul(
                            gp[:], wgb_t[:, jj, :], pms[:], start=True, stop=True
                        )
                        nc.scalar.activation(
                            gs[:], gp[:], AF.Sigmoid, bias=bgb_t[:, jj : jj + 1]
                        )
                        gates.append(gs)
                        if stage == "mlp" and j == 0:
                            nc.sync.dma_start(
                                dbg.ap()[
                                    :,
                                    RW + 256 + jj * 128 : RW + 256 + (jj + 1) * 128,
                                ],
                                gs[:],
                            )
                    if stage == "mlp":
                        return

                    # value path: one transpose per component covers both
                    # tiles of the pair:
                    # in [128 q, (2 t x 64 d)] -> out [(2 t x 64 d), 128 q]
                    vstage = small.tile([128, 3, 3, 128], f32, tag="vstage")
                    if NOVAL:
                        nc.vector.memset(
                            vstage[:].rearrange("p a b c -> p (a b c)"), 0.25
                        )
                    for c in range(0 if NOVAL else 3):
                        for jj in range(3):
                            vp = pssm.tile([128, 128], f32, tag="pssm")
                            nc.tensor.matmul(
                                vp[:], wvb_t[:, jj, :], aggs3[:, c, :],
                                start=True, stop=True,
                            )
                            TT(vstage[:, c, jj, :], vp[:], gates[jj][:], ALU.mult)

                    for h in range(2):
                        q0 = (2 * j + h) * 128
                        if SIMPLEOUT:
                            nc.sync.dma_start(
                                out_dev.ap()[0:64, q0 * 9 : q0 * 9 + 9 * 128],
                                vstage[h * 64 : (h + 1) * 64, :, :, :].rearrange(
                                    "p c jj q -> p (c jj q)"
                                ),
                            )
                        else:
                            nc.gpsimd.dma_start(
                                out_re[:, :, :, q0 : q0 + 128].rearrange(
                                    "p c jj q -> p (c jj) q"
                                ),
                                vstage[h * 64 : (h + 1) * 64, :, :, :].rearrange(
                                    "p c jj q -> p (c jj) q"
                                ),
                            )

                if stage == "mlponly":
                    nc.vector.memset(
                        p4[:].rearrange("p t c k -> p (t c k)"), 0.25
                    )
                    for j in range(NPAIR):
                        emit_pair(j)
                elif stage == "ppf":
                    emit_ppf(0, T)
                    for ci in range(4):
                        nc.sync.dma_start(
                            dbg.ap()[:, ci * FW : (ci + 1) * FW], plane_view(ci)
                        )
                elif stage == "mlp":
                    emit_ppf(0, T)
                    emit_pair(0)
                else:
                    CT = int(os.environ.get("BENCH_CT", "20"))
                    assert T % CT == 0 and CT % 2 == 0
                    for t0 in range(0, T, CT):
                        emit_ppf(t0, CT)
                        for j in range(t0 // 2, (t0 + CT) // 2):
                            emit_pair(j)

    nc.compile()
    return nc


def _f32_to_bf16_bits(x):
    """Round-to-nearest-even f32 -> bf16, returned as uint16 bits."""
    u = np.ascontiguousarray(x, dtype=np.float32).view(np.uint32)
    rounded = (u + 0x7FFF + ((u >> 16) & 1)) >> 16
    return rounded.astype(np.uint16)


def _host_prep(q_pts, s_pts, s_feats, neighbor_indices, normals,
               W1, b1, W2, b2, W3, b3, Wg, bg, Wv, T, n_total=N):
    NQ = 128 * T
    n_per_core = n_total // N_CORES
    f = np.float32

    comb = np.zeros((M, ES), dtype=f)
    cb = comb.view(np.uint16).reshape(M, ES * 2)
    cb[:, : 2 * SFW] = _f32_to_bf16_bits(s_feats.reshape(M, 192))
    comb[:, PNO : PNO + 3] = s_pts
    comb[:, PNO + 3 : PNO + 6] = normals

    W1T = W1.T.astype(f).copy()
    W1T[1:4] *= f(1.0 / PI)
    w1b = np.zeros((8, 128), dtype=f)
    w1b[0:4, 0:64] = W1T
    w1b[4:8, 64:128] = W1T
    b1b = np.concatenate([b1, b1]).astype(f)[:, None]

    def blockdiag2(A):
        n_, m_ = A.shape
        o = np.zeros((2 * n_, 2 * m_), dtype=f)
        o[:n_, :m_] = A
        o[n_:, m_:] = A
        return o

    w2b = blockdiag2(W2.T.astype(f))
    b2b = np.concatenate([b2, b2]).astype(f)[:, None]
    w3b = blockdiag2((W3.T / K).astype(f))
    b3b = np.concatenate([b3, b3]).astype(f)[:, None]

    WgT = Wg.T.astype(f)
    WvT = (Wv.T / K).astype(f)
    wgb = np.zeros((3, 128, 128), dtype=f)
    wvb = np.zeros((3, 128, 128), dtype=f)
    bgb = np.zeros((128, 3), dtype=f)
    for jj in range(3):
        wgb[jj] = blockdiag2(WgT[:, jj * 64 : (jj + 1) * 64])
        wvb[jj] = blockdiag2(WvT[:, jj * 64 : (jj + 1) * 64])
        bgb[:, jj] = np.concatenate([bg[jj * 64 : (jj + 1) * 64]] * 2)
    wgb_host = np.ascontiguousarray(wgb.transpose(1, 0, 2))
    wvb_host = np.ascontiguousarray(wvb.transpose(1, 0, 2))
    ident = np.eye(128, dtype=f)

    shared = dict(
        comb=comb, w1b=w1b.astype(np.float16), b1b=b1b,
        w2b=w2b.astype(np.float16), b2b=b2b, w3b=w3b, b3b=b3b,
        wgb=wgb_host, bgb=bgb, wvb=wvb_host, ident=ident,
    )

    in_maps = []
    for i in range(N_CORES):
        lo = i * n_per_core
        hi = lo + n_per_core
        qp_pad = np.zeros((NQ, 3), dtype=f)
        qp_pad[: hi - lo] = q_pts[lo:hi]
        idx_pad = np.zeros((NQ, K), dtype=np.int64)
        idx_pad[: hi - lo] = neighbor_indices[lo:hi]

        qp_host = np.ascontiguousarray(
            qp_pad.reshape(T, 128, 3).transpose(1, 0, 2)
        ).astype(np.float16)

        # idx16[p, t, g, s]: gather g of tile t covers logical rows
        # i' = (k - g*KPG)*128 + q, wrapped: w[l, s] = list[s*16 + l]
        idx16 = np.zeros((128, T, GPT, NI // 16), np.int16)
        for t in range(T):
            arr = idx_pad[t * 128 : (t + 1) * 128, :]      # [128 q, K]
            for g in range(GPT):
                lst = arr[:, g * KPG : (g + 1) * KPG].T.reshape(NI)
                idx16[:, t, g, :] = np.tile(
                    lst.reshape(NI // 16, 16).T.astype(np.int16), (8, 1)
                )

        m = dict(shared)
        m.update(qp=qp_host, idx16=idx16)
        in_maps.append(m)
    return in_maps


def kernel(**inputs):
    from concourse.bass_utils import run_bass_kernel_spmd

    T = 20
    inputs = {k: np.asarray(v) for k, v in inputs.items()}
    idx = inputs["neighbor_indices"].astype(np.int64)

    if T not in _NC_CACHE:
        _NC_CACHE[T] = _build_nc(T)
    nc = _NC_CACHE[T]

    in_maps = _host_prep(
        inputs["q_pts"], inputs["s_pts"], inputs["s_feats"], idx,
        inputs["normals"], inputs["W1"], inputs["b1"], inputs["W2"],
        inputs["b2"], inputs["W3"], inputs["b3"], inputs["Wg"],
        inputs["bg"], inputs["Wv"], T,
    )
    res = run_bass_kernel_spmd(nc, in_maps, core_ids=list(range(N_CORES)))

    n_per_core = N // N_CORES
    out = np.empty((N, OUT, 3), dtype=np.float32)
    for i in range(N_CORES):
        o = res.results[i]["out"]
        out[i * n_per_core : (i + 1) * n_per_core] = o.transpose(2, 1, 0)[:n_per_core]
    return out



# revision 10
# speedup vs baseline: 1.9765x; 1.9765x over previous
"""Trainium2 Bass kernel for EquivariantPPFAttention (gnn_message_passing).

Contract: kernel(**inputs) takes FULL unsharded inputs (as produced by
reference.setup_inputs()) and returns the FULL [N, OUT, 3] float32 output.

Strategy (data-parallel over query points N across 8 NeuronCores):
  - shard q_pts / neighbor_indices across cores; replicate everything else.
  - one combined gather table comb[M, 512B]: s_feats row in bf16 (384B) +
    s_pts/normals in f32 (24B) + pad. Per query-tile of 128, dma_gather
    pulls all 128*32 neighbor rows (4 gathers of 1024 idxs - HW limit).
  - K-sum of the bf16 s_feats part on DVE (strided reduce, f32 accum);
    pts/normals extracted to a packed buffer for the PPF phase.
  - PPF angles via DVE arithmetic + ACT Sqrt/Arctan
    (atan2(r,y) = atan(r/y) + pi*[y<0] for r>=0); the four PPF features
    are written interleaved into one p4[:, t, ci, k] tensor.
  - tiny MLP on TensorE with rows on the free dim; two query-tiles packed
    per matmul via block-diagonal weights; mean-over-K folded into W3; the
    1/pi PPF normalization folded into W1; 1/K of the value path folded
    into Wv.
  - MLP input pack: per tile one PE transpose of p4 ([128 q, (ci k)] ->
    [(ci k), 128 q]) then ONE 4-row DMA (dst [4,4096] row-major ==
    src [128,128] partition-major traversal), alternating SP/ACT/Pool
    issue queues so no single sequencer serializes the pack; pf columns
    are k-major (col=k*128+q), so the K-mean reduce uses a strided view.
    h1/h2 run in f16 (weights cast on host); after the K-sum all f32.
    PSUM->SBUF staging copies ride the ACT engine to keep DVE free.
  - output DMAs issue from the Pool queue to keep the SP queue free for
    pack DMAs (in-order sequencers otherwise stall the next pair's pack).
"""

import math
import os
import numpy as np

N = 20000
M = 20000
K = 32
D = 64
HID = 64
OUT = 192
PPF_OUT = 64
N_CORES = 8
PI = math.pi

ES = 128          # f32 elems per comb row (512 B)
SFW = 96          # f32 slots holding the 192 bf16 s_feats values
PNO = 96          # f32 slot offset of pts/normals (6 floats)
NI = 1024         # idxs per dma_gather (HW-stable limit)
GPT = (128 * K) // NI   # gathers per query tile (4)
KPG = K // GPT    # k-blocks per gather (8)

_NC_CACHE = {}


def _build_nc(T, stage="full", loop=None):
    """Per-core Bass program for T query-tiles of 128.

    stage: debug bisection point - gather | ppf | mlp | full.
    loop: if set, repeat the whole body N times (for timing).
    """
    from contextlib import ExitStack, nullcontext
    from concourse import bacc, bass, mybir, tile

    assert T % 2 == 0
    NPAIR = T // 2
    NQ = 128 * T
    f32 = mybir.dt.float32
    bf16 = mybir.dt.bfloat16
    f16 = mybir.dt.float16
    i16 = mybir.dt.int16
    AF = mybir.ActivationFunctionType
    ALU = mybir.AluOpType

    nc = bacc.Bacc(
        "TRN2", target_bir_lowering=False, debug=False, num_swdge_queues=4
    )

    comb_in = nc.dram_tensor("comb", [M, ES], f32, kind="ExternalInput")
    qp_in = nc.dram_tensor("qp", [128, T, 3], f16, kind="ExternalInput")
    idx_in = nc.dram_tensor("idx16", [128, T, GPT, NI // 16], i16,
                            kind="ExternalInput")
    w1b_in = nc.dram_tensor("w1b", [8, 128], f16, kind="ExternalInput")
    b1b_in = nc.dram_tensor("b1b", [128, 1], f32, kind="ExternalInput")
    w2b_in = nc.dram_tensor("w2b", [128, 128], f16, kind="ExternalInput")
    b2b_in = nc.dram_tensor("b2b", [128, 1], f32, kind="ExternalInput")
    w3b_in = nc.dram_tensor("w3b", [128, 128], f32, kind="ExternalInput")
    b3b_in = nc.dram_tensor("b3b", [128, 1], f32, kind="ExternalInput")
    wgb_in = nc.dram_tensor("wgb", [128, 3, 128], f32, kind="ExternalInput")
    bgb_in = nc.dram_tensor("bgb", [128, 3], f32, kind="ExternalInput")
    wvb_in = nc.dram_tensor("wvb", [128, 3, 128], f32, kind="ExternalInput")
    ident_in = nc.dram_tensor("ident", [128, 128], f32, kind="ExternalInput")

    SIMPLEOUT = bool(int(os.environ.get("BENCH_SIMPLEOUT", "0")))
    if stage in ("full", "mlponly"):
        if SIMPLEOUT:
            out_dev = nc.dram_tensor(
                "out", [128, 9 * NQ], f32, kind="ExternalOutput"
            )
        else:
            out_dev = nc.dram_tensor(
                "out", [3, OUT, NQ], f32, kind="ExternalOutput"
            )
        dbg = None
    else:
        DBGW = {
            "gather": T * K * 8 + T * 192,
            "ppf": 4 * T * K,
            "mlp": 128 * K + 128 + 128 + 3 * 128,
            "gonly": T * 8,
            "gred": T * 8,
            "dbgdma": T * K * 8 + T * 192,
        }[stage]
        out_dev = None
        dbg = nc.dram_tensor("dbg", [128, DBGW], f32, kind="ExternalOutput")

    with tile.TileContext(nc) as tc, ExitStack() as ctx:
        const = ctx.enter_context(tc.tile_pool(name="const", bufs=1))
        gpool = ctx.enter_context(tc.tile_pool(name="gpool", bufs=2))
        gath = ctx.enter_context(tc.tile_pool(name="gath", bufs=1))
        planes = ctx.enter_context(tc.tile_pool(name="planes", bufs=1))
        temps = ctx.enter_context(tc.tile_pool(name="temps", bufs=2))
        mlpp = ctx.enter_context(tc.tile_pool(name="mlpp", bufs=1))
        small = ctx.enter_context(tc.tile_pool(name="small", bufs=2))
        psmlp = ctx.enter_context(tc.tile_pool(name="psmlp", bufs=3, space="PSUM"))
        pssm = ctx.enter_context(tc.tile_pool(name="pssm", bufs=3, space="PSUM"))
        pstp = ctx.enter_context(tc.tile_pool(name="pstp", bufs=2, space="PSUM"))

        def cload(name, dram, shape, dt=f32):
            t = const.tile(shape, dt, tag=name, name=name)
            if len(shape) > 3:
                dims = " ".join(f"d{i}" for i in range(len(shape) - 1))
                pat = f"p {dims} -> p ({dims})"
                nc.sync.dma_start(t[:].rearrange(pat), dram.ap().rearrange(pat))
            else:
                nc.sync.dma_start(t[:], dram.ap())
            return t

        qp_t = cload("qp", qp_in, [128, T, 3], f16)
        idx_t = cload("idx16", idx_in, [128, T, GPT, NI // 16], i16)
        w1b_t = cload("w1b", w1b_in, [8, 128], f16)
        b1b_t = cload("b1b", b1b_in, [128, 1])
        w2b_t = cload("w2b", w2b_in, [128, 128], f16)
        b2b_t = cload("b2b", b2b_in, [128, 1])
        w3b_t = cload("w3b", w3b_in, [128, 128])
        b3b_t = cload("b3b", b3b_in, [128, 1])
        wgb_t = cload("wgb", wgb_in, [128, 3, 128])
        bgb_t = cload("bgb", bgb_in, [128, 3])
        wvb_t = cload("wvb", wvb_in, [128, 3, 128])
        ident_t = cload("ident", ident_in, [128, 128])

        _loop_ctx = tc.For_i(0, loop, 1) if loop else nullcontext()
        with _loop_ctx:
            # ---- gather + per-tile K-reduce + pn extraction ----
            nbbuf = gath.tile([128, T, K, 8], f16, tag="nbbuf")
            sfsum = gath.tile([128, T, 192], f32, tag="sfsum")

            do_gather = stage not in ("dbgdma", "mlponly")
            do_reduce = stage not in ("gonly", "dbgdma", "mlponly")

            if stage in ("dbgdma", "mlponly"):
                nc.vector.memset(nbbuf[:].rearrange("p t k c -> p (t k c)"), 0.25)
                nc.vector.memset(sfsum[:].rearrange("p t c -> p (t c)"), 0.25)

            NORED = bool(int(os.environ.get("BENCH_NORED", "0")))
            if NORED and stage == "full":
                nc.vector.memset(nbbuf[:].rearrange("p t k c -> p (t k c)"), 0.25)
                nc.vector.memset(sfsum[:].rearrange("p t c -> p (t c)"), 0.25)

            gt_last = None
            if do_gather:
                for t in range(T):
                    gt = gpool.tile([128, K, ES], f32, tag="gt", name="gt")
                    gt_last = gt
                    for g in range(GPT):
                        # queue g -> Q7 core pair g: the 4 gathers of one
                        # tile run concurrently on the 4 SWDGE core pairs.
                        nc.gpsimd.dma_gather(
                            out_ap=gt[:, g * KPG : (g + 1) * KPG, :],
                            in_ap=comb_in.ap(),
                            idxs_ap=idx_t[:, t, g, :],
                            num_idxs=NI,
                            num_idxs_reg=NI,
                            elem_size=ES,
                            queue_num=g,
                        )
                    if do_reduce and not NORED:
                        # K-sum of the bf16 s_feats block via a contiguous
                        # tree-add (f16 intermediates, f32 final level)
                        gtb = gt[:].bitcast(bf16)          # [128, K, 256]
                        ga = gpool.tile([128, 16, 256], f16, tag="ga", bufs=2)
                        nc.vector.tensor_tensor(
                            ga[:, :, 0:192], gtb[:, 0:16, 0:192],
                            gtb[:, 16:32, 0:192], ALU.add,
                        )
                        for lv in (8, 4, 2):
                            nc.vector.tensor_tensor(
                                ga[:, 0:lv, 0:192], ga[:, 0:lv, 0:192],
                                ga[:, lv : 2 * lv, 0:192], ALU.add,
                            )
                        nc.vector.tensor_tensor(
                            sfsum[:, t, :], ga[:, 0, 0:192], ga[:, 1, 0:192],
                            ALU.add,
                        )
                        # pts/normals (f32) -> nbbuf[:, t, :, 0:6]
                        nc.scalar.activation(
                            nbbuf[:, t, :, 0:6], gt[:, :, PNO : PNO + 6], AF.Copy
                        )
            if stage in ("gonly", "gred"):
                sb = gath.tile([128, T * 8], f32, tag="sdbg")
                nc.vector.tensor_copy(
                    sb[:],
                    nbbuf[:, 0, :, :].rearrange("p k c -> p (k c)")[:, : T * 8],
                )
                nc.sync.dma_start(dbg.ap()[:, : T * 8], sb[:])
            elif stage in ("gather", "dbgdma"):
                nc.sync.dma_start(
                    dbg.ap()[:, : T * K * 8],
                    nbbuf[:].rearrange("p t k c -> p (t k c)"),
                )
                nc.sync.dma_start(
                    dbg.ap()[:, T * K * 8 :],
                    sfsum[:].rearrange("p t c -> p (t c)"),
                )
            else:
                # ---- PPF + MLP, emitted in tile chunks so gather/PPF of
                # chunk c+1 overlaps the MLP pipeline of chunk c ----
                FW = T * K
                TT = nc.vector.tensor_tensor
                STT = nc.vector.scalar_tensor_tensor

                # interleaved PPF planes: p4[:, t, ci, k] (ci-major per tile)
                p4 = planes.tile([128, T, 4, K], f32, tag="p4", name="p4")

                def plane_view(ci):
                    # [128, FW] view of feature ci (strided; debug stages only)
                    return p4[:, :, ci, :].rearrange("p t k -> p (t k)")

                RW = 128 * K  # rows per query-tile (4096)
                HC = RW // 2
                NOPACK = bool(int(os.environ.get("BENCH_NOPACK", "0")))
                NOVAL = bool(int(os.environ.get("BENCH_NOVAL", "0")))
                NOGATE = bool(int(os.environ.get("BENCH_NOGATE", "0")))
                NOMLP12 = bool(int(os.environ.get("BENCH_NOMLP12", "0")))
                V1PACK = bool(int(os.environ.get("BENCH_V1PACK", "0")))
                PACKNODMA = bool(int(os.environ.get("BENCH_PACKNODMA", "0")))
                PACKNOTP = bool(int(os.environ.get("BENCH_PACKNOTP", "0")))

                if stage in ("full", "mlponly") and not SIMPLEOUT:
                    out_re = out_dev.ap().rearrange(
                        "c (jj p) q -> p c jj q", jj=3
                    )

                def emit_ppf(t0, tn):
                    with nc.allow_low_precision(reason="f16 PPF chain"):
                        _emit_ppf(t0, tn)

                def _emit_ppf(t0, tn):
                    # |a x b|^2 = |a|^2 |b|^2 - (a.b)^2 : no cross products.
                    nb = nbbuf[:, t0 : t0 + tn]
                    TS = nc.vector.tensor_scalar

                    def ttile(tag):
                        return temps.tile([128, tn * K], f16, tag=tag, name=tag)

                    def v3(t_):
                        return t_[:].rearrange("p (t k) -> p t k", k=K)

                    npv = [nb[:, :, :, c] for c in range(3)]
                    nnv = [nb[:, :, :, 3 + c] for c in range(3)]
                    qnv = [
                        nb[:, :, 0, 3 + c].to_broadcast([128, tn, K])
                        for c in range(3)
                    ]
                    qpv = [
                        qp_t[:, t0 : t0 + tn, c].to_broadcast([128, tn, K])
                        for c in range(3)
                    ]

                    vd = []
                    for c in range(3):
                        t_ = ttile(f"vd{c}")
                        TT(v3(t_), npv[c], qpv[c], ALU.subtract)
                        vd.append(t_)

                    def dot_views(av, bv, out_tag):
                        m0 = ttile("dm0")
                        TT(m0[:], av[0], bv[0], ALU.mult)
                        m1 = ttile("dm1")
                        TT(m1[:], av[1], bv[1], ALU.mult)
                        s = ttile(out_tag)
                        TT(s[:], m0[:], m1[:], ALU.add)
                        m2 = ttile("dm0")
                        TT(m2[:], av[2], bv[2], ALU.mult)
                        TT(s[:], s[:], m2[:], ALU.add)
                        return s

                    vdv = [v3(t_) for t_ in vd]

                    dd = dot_views(vdv, vdv, "dd")
                    y1 = dot_views(qnv, vdv, "y1")
                    y2 = dot_views(nnv, vdv, "y2")
                    y3 = dot_views(qnv, nnv, "y3")
                    nnsq = dot_views(nnv, nnv, "nnsq")

                    # |qn|^2 per query: [128, tn] (k=0 slice), ~free
                    qn_sq = temps.tile([128, tn], f16, tag="qnsq", name="qnsq")
                    qm = temps.tile([128, tn], f16, tag="qm")
                    TT(qn_sq[:], nb[:, :, 0, 3], nb[:, :, 0, 3], ALU.mult)
                    TT(qm[:], nb[:, :, 0, 4], nb[:, :, 0, 4], ALU.mult)
                    TT(qn_sq[:], qn_sq[:], qm[:], ALU.add)
                    TT(qm[:], nb[:, :, 0, 5], nb[:, :, 0, 5], ALU.mult)
                    TT(qn_sq[:], qn_sq[:], qm[:], ALU.add)
                    qnsq_b = qn_sq[:].to_broadcast([128, tn, K])

                    ddv = v3(dd)
                    nnsqv = v3(nnsq)

                    # msq_i = |a|^2 |b|^2 ; rs_i = max(msq_i - y_i^2, 0)
                    ys = [y1, y2, y3]
                    rss = []
                    for ci in range(3):
                        msq = ttile(f"msq{ci}")
                        if ci == 0:
                            TT(v3(msq), qnsq_b, ddv, ALU.mult)
                        elif ci == 1:
                            TT(v3(msq), nnsqv, ddv, ALU.mult)
                        else:
                            TT(v3(msq), qnsq_b, nnsqv, ALU.mult)
                        ysq = ttile("ysq")
                        TT(ysq[:], ys[ci][:], ys[ci][:], ALU.mult)
                        TT(msq[:], msq[:], ysq[:], ALU.subtract)
                        TS(msq[:], msq[:], 0.0, None, ALU.max)
                        rss.append(msq)

                    # ACT pass: Sqrt x4 (one table set), Reciprocal x3,
                    # then Arctan x3 (shares table with Relu/Sigmoid).
                    nc.scalar.activation(
                        p4[:, t0 : t0 + tn, 0, :], ddv, AF.Sqrt
                    )
                    rs_r = []
                    for ci in range(3):
                        r = ttile(f"r{ci}")
                        nc.scalar.activation(r[:], rss[ci][:], AF.Sqrt)
                        rs_r.append(r)
                    iys = []
                    for ci in range(3):
                        iy = ttile(f"iy{ci}")
                        nc.vector.reciprocal(iy[:], ys[ci][:])
                        iys.append(iy)

                    for ci in range(3):
                        tq = ttile("dm0")
                        TT(tq[:], rs_r[ci][:], iys[ci][:], ALU.mult)
                        at = ttile("dm1")
                        nc.scalar.activation(at[:], tq[:], AF.Arctan)
                        ind = ttile("ind")
                        TS(ind[:], ys[ci][:], 0.0, None, ALU.is_lt)
                        STT(
                            p4[:, t0 : t0 + tn, ci + 1, :],
                            v3(ind), PI, v3(at), ALU.mult, ALU.add,
                        )

                def emit_pair(j):
                    pf = mlpp.tile([8, RW], f16, tag="pf", bufs=2)
                    if NOPACK:
                        nc.vector.memset(pf[:], 0.25)
                    elif V1PACK:
                        for t2 in range(2):
                            t_abs = 2 * j + t2
                            for ci in range(4):
                                nc.sync.dma_start(
                                    pf[t2 * 4 + ci : t2 * 4 + ci + 1, :],
                                    p4[:, t_abs, ci, :],
                                )
                    else:
                        # pack via PE transpose: p4 tile slab [128 q, (ci k)]
                        # -> [(ci k), 128 q]; then 4 row-DMAs per tile with
                        # 512B descriptors, spread over 3 DMA-issue engines.
                        pts_s = mlpp.tile([128, 2, 128], f16, tag="pts", bufs=3)
                        if PACKNOTP:
                            nc.vector.memset(
                                pts_s[:].rearrange("p t q -> p (t q)"), 0.25
                            )
                        else:
                            for t2 in range(2):
                                t_abs = 2 * j + t2
                                tp = pstp.tile([128, 128], f32, tag="pstp")
                                nc.tensor.transpose(
                                    tp[:],
                                    p4[:, t_abs, :, :].rearrange(
                                        "p c k -> p (c k)"
                                    ),
                                    ident_t[:],
                                )
                                nc.scalar.activation(
                                    pts_s[:, t2, :], tp[:], AF.Copy
                                )
                        if PACKNODMA:
                            nc.vector.memset(pf[:], 0.25)
                        else:
                            # one DMA per tile: dst [4, 4096] row-major ==
                            # src [128, 128] partition-major traversal
                            # (dst col k*128+q <- src partition ci*32+k col q).
                            # Pool queues are reserved for the gathers now, so
                            # packs ride the two HWDGE rings (SP/ACT).
                            engs = [nc.sync, nc.scalar]
                            for t2 in range(2):
                                eng = engs[(j * 2 + t2) % 2]
                                eng.dma_start(
                                    pf[t2 * 4 : (t2 + 1) * 4, :],
                                    pts_s[:, t2, :],
                                )

                    # value-path transposes depend only on sfsum: run them
                    # on PE/ACT while the h1/h2 stream occupies the pipeline
                    if stage != "mlp" and not NOVAL:
                        av_e = sfsum[:, 2 * j : 2 * j + 2, :].rearrange(
                            "p t (d c) -> p c (t d)", c=3
                        )
                        aggs3 = small.tile([128, 3, 128], f32, tag="aggs3")
                        for c in range(3):
                            tpv = pssm.tile([128, 128], f32, tag="pssm")
                            nc.tensor.transpose(tpv[:], av_e[:, c, :], ident_t[:])
                            nc.scalar.activation(aggs3[:, c, :], tpv[:], AF.Copy)

                    ksum = small.tile([128, 128], f32, tag="ksum")
                    if NOMLP12:
                        nc.vector.memset(ksum[:], 0.25)
                    kparts = []
                    for hh in range(0 if NOMLP12 else 2):
                        h1s = mlpp.tile([128, HC], f16, tag="h1s", bufs=2)
                        for ch in range(HC // 512):
                            sl = slice(ch * 512, (ch + 1) * 512)
                            slg = slice(
                                hh * HC + ch * 512, hh * HC + (ch + 1) * 512
                            )
                            h1p = psmlp.tile([128, 512], f32, tag="psmlp")
                            nc.tensor.matmul(
                                h1p[:], w1b_t[:], pf[:, slg], start=True, stop=True
                            )
                            nc.scalar.activation(
                                h1s[:, sl], h1p[:], AF.Relu, bias=b1b_t[:]
                            )
                        h2s = mlpp.tile([128, HC], f16, tag="h2s", bufs=2)
                        for ch in range(HC // 512):
                            sl = slice(ch * 512, (ch + 1) * 512)
                            h2p = psmlp.tile([128, 512], f32, tag="psmlp")
                            nc.tensor.matmul(
                                h2p[:], w2b_t[:], h1s[:, sl], start=True, stop=True
                            )
                            if ch % 2 == 0:
                                nc.vector.tensor_scalar(
                                    h2s[:, sl], h2p[:], b2b_t[:], 0.0,
                                    ALU.add, ALU.max,
                                )
                            else:
                                nc.scalar.activation(
                                    h2s[:, sl], h2p[:], AF.Relu, bias=b2b_t[:]
                                )
                        # cols are k-major (col = k*128 + q): each hh holds
                        # k-block hh*16..hh*16+15 for all 128 queries.
                        # K-reduce via f16 tree adds (2x DVE mode) instead of
                        # tensor_reduce (1x mode).
                        h2v = h2s[:].rearrange("p (k q) -> p k q", q=128)
                        for lv in (8, 4, 2):
                            TT(
                                h2v[:, 0:lv, :], h2v[:, 0:lv, :],
                                h2v[:, lv : 2 * lv, :], ALU.add,
                            )
                        kp = small.tile([128, 128], f32, tag=f"kp{hh}")
                        TT(kp[:], h2v[:, 0, :], h2v[:, 1, :], ALU.add)
                        kparts.append(kp)
                    if not NOMLP12:
                        TT(ksum[:], kparts[0][:], kparts[1][:], ALU.add)

                    pmp = pssm.tile([128, 128], f32, tag="pssm")
                    nc.tensor.matmul(pmp[:], w3b_t[:], ksum[:], start=True, stop=True)
                    pms = small.tile([128, 128], f32, tag="pms")
                    nc.vector.tensor_scalar_add(pms[:], pmp[:], b3b_t[:])

                    if stage == "mlp" and j == 0:
                        nc.sync.dma_start(dbg.ap()[:, : RW // 2], h2s[:])
                        nc.sync.dma_start(dbg.ap()[:, RW : RW + 128], ksum[:])
                        nc.sync.dma_start(dbg.ap()[:, RW + 128 : RW + 256], pms[:])

                    gates = []
                    for jj in range(3):
                        gs = small.tile(
                            [128, 128], f32, tag=f"gate{jj}", name=f"gate{jj}"
                        )
                        if NOGATE:
                            nc.vector.memset(gs[:], 0.5)
                            gates.append(gs)
                            continue
                        gp = pssm.tile([128, 128], f32, tag="pssm")
                        nc.tensor.matmul(
                            gp[:], wgb_t[:, jj, :], pms[:], start=True, stop=True
                        )
                        nc.scalar.activation(
                            gs[:], gp[:], AF.Sigmoid, bias=bgb_t[:, jj : jj + 1]
                        )
                        gates.append(gs)
                        if stage == "mlp" and j == 0:
                            nc.sync.dma_start(
                                dbg.ap()[
                                    :,
                                    RW + 256 + jj * 128 : RW + 256 + (jj + 1) * 128,
                                ],
                                gs[:],
                            )
                    if stage == "mlp":
                        return

                    # value path: one transpose per component covers both
                    # tiles of the pair:
                    # in [128 q, (2 t x 64 d)] -> out [(2 t x 64 d), 128 q]
                    vstage = small.tile([128, 3, 3, 128], f32, tag="vstage")
                    if NOVAL:
                        nc.vector.memset(
                            vstage[:].rearrange("p a b c -> p (a b c)"), 0.25
                        )
                    for c in range(0 if NOVAL else 3):
                        for jj in range(3):
                            vp = pssm.tile([128, 128], f32, tag="pssm")
                            nc.tensor.matmul(
                                vp[:], wvb_t[:, jj, :], aggs3[:, c, :],
                                start=True, stop=True,
                            )
                            TT(vstage[:, c, jj, :], vp[:], gates[jj][:], ALU.mult)

                    for h in range(2):
                        q0 = (2 * j + h) * 128
                        if SIMPLEOUT:
                            nc.sync.dma_start(
                                out_dev.ap()[0:64, q0 * 9 : q0 * 9 + 9 * 128],
                                vstage[h * 64 : (h + 1) * 64, :, :, :].rearrange(
                                    "p c jj q -> p (c jj q)"
                                ),
                            )
                        else:
                            eng = nc.scalar if h == 0 else nc.sync
                            eng.dma_start(
                                out_re[:, :, :, q0 : q0 + 128].rearrange(
                                    "p c jj q -> p (c jj) q"
                                ),
                                vstage[h * 64 : (h + 1) * 64, :, :, :].rearrange(
                                    "p c jj q -> p (c jj) q"
                                ),
                            )

                if stage == "mlponly":
                    nc.vector.memset(
                        p4[:].rearrange("p t c k -> p (t c k)"), 0.25
                    )
                    for j in range(NPAIR):
                        emit_pair(j)
                elif stage == "ppf":
                    emit_ppf(0, T)
                    for ci in range(4):
                        nc.sync.dma_start(
                            dbg.ap()[:, ci * FW : (ci + 1) * FW], plane_view(ci)
                        )
                elif stage == "mlp":
                    emit_ppf(0, T)
                    emit_pair(0)
                else:
                    CT = int(os.environ.get("BENCH_CT", "4"))
                    assert T % CT == 0 and CT % 2 == 0
                    for t0 in range(0, T, CT):
                        emit_ppf(t0, CT)
                        for j in range(t0 // 2, (t0 + CT) // 2):
                            emit_pair(j)

    nc.compile()
    return nc


def _f32_to_bf16_bits(x):
    """Round-to-nearest-even f32 -> bf16, returned as uint16 bits."""
    u = np.ascontiguousarray(x, dtype=np.float32).view(np.uint32)
    rounded = (u + 0x7FFF + ((u >> 16) & 1)) >> 16
    return rounded.astype(np.uint16)


def _host_prep(q_pts, s_pts, s_feats, neighbor_indices, normals,
               W1, b1, W2, b2, W3, b3, Wg, bg, Wv, T, n_total=N):
    NQ = 128 * T
    n_per_core = n_total // N_CORES
    f = np.float32

    comb = np.zeros((M, ES), dtype=f)
    cb = comb.view(np.uint16).reshape(M, ES * 2)
    cb[:, : 2 * SFW] = _f32_to_bf16_bits(s_feats.reshape(M, 192))
    comb[:, PNO : PNO + 3] = s_pts
    comb[:, PNO + 3 : PNO + 6] = normals

    W1T = W1.T.astype(f).copy()
    W1T[1:4] *= f(1.0 / PI)
    w1b = np.zeros((8, 128), dtype=f)
    w1b[0:4, 0:64] = W1T
    w1b[4:8, 64:128] = W1T
    b1b = np.concatenate([b1, b1]).astype(f)[:, None]

    def blockdiag2(A):
        n_, m_ = A.shape
        o = np.zeros((2 * n_, 2 * m_), dtype=f)
        o[:n_, :m_] = A
        o[n_:, m_:] = A
        return o

    w2b = blockdiag2(W2.T.astype(f))
    b2b = np.concatenate([b2, b2]).astype(f)[:, None]
    w3b = blockdiag2((W3.T / K).astype(f))
    b3b = np.concatenate([b3, b3]).astype(f)[:, None]

    WgT = Wg.T.astype(f)
    WvT = (Wv.T / K).astype(f)
    wgb = np.zeros((3, 128, 128), dtype=f)
    wvb = np.zeros((3, 128, 128), dtype=f)
    bgb = np.zeros((128, 3), dtype=f)
    for jj in range(3):
        wgb[jj] = blockdiag2(WgT[:, jj * 64 : (jj + 1) * 64])
        wvb[jj] = blockdiag2(WvT[:, jj * 64 : (jj + 1) * 64])
        bgb[:, jj] = np.concatenate([bg[jj * 64 : (jj + 1) * 64]] * 2)
    wgb_host = np.ascontiguousarray(wgb.transpose(1, 0, 2))
    wvb_host = np.ascontiguousarray(wvb.transpose(1, 0, 2))
    ident = np.eye(128, dtype=f)

    shared = dict(
        comb=comb, w1b=w1b.astype(np.float16), b1b=b1b,
        w2b=w2b.astype(np.float16), b2b=b2b, w3b=w3b, b3b=b3b,
        wgb=wgb_host, bgb=bgb, wvb=wvb_host, ident=ident,
    )

    in_maps = []
    for i in range(N_CORES):
        lo = i * n_per_core
        hi = lo + n_per_core
        qp_pad = np.zeros((NQ, 3), dtype=f)
        qp_pad[: hi - lo] = q_pts[lo:hi]
        idx_pad = np.zeros((NQ, K), dtype=np.int64)
        idx_pad[: hi - lo] = neighbor_indices[lo:hi]

        qp_host = np.ascontiguousarray(
            qp_pad.reshape(T, 128, 3).transpose(1, 0, 2)
        ).astype(np.float16)

        # idx16[p, t, g, s]: gather g of tile t covers logical rows
        # i' = (k - g*KPG)*128 + q, wrapped: w[l, s] = list[s*16 + l]
        idx16 = np.zeros((128, T, GPT, NI // 16), np.int16)
        for t in range(T):
            arr = idx_pad[t * 128 : (t + 1) * 128, :]      # [128 q, K]
            for g in range(GPT):
                lst = arr[:, g * KPG : (g + 1) * KPG].T.reshape(NI)
                idx16[:, t, g, :] = np.tile(
                    lst.reshape(NI // 16, 16).T.astype(np.int16), (8, 1)
                )

        m = dict(shared)
        m.update(qp=qp_host, idx16=idx16)
        in_maps.append(m)
    return in_maps


def kernel(**inputs):
    from concourse.bass_utils import run_bass_kernel_spmd

    T = 20
    inputs = {k: np.asarray(v) for k, v in inputs.items()}
    idx = inputs["neighbor_indices"].astype(np.int64)

    if T not in _NC_CACHE:
        _NC_CACHE[T] = _build_nc(T)
    nc = _NC_CACHE[T]

    in_maps = _host_prep(
        inputs["q_pts"], inputs["s_pts"], inputs["s_feats"], idx,
        inputs["normals"], inputs["W1"], inputs["b1"], inputs["W2"],
        inputs["b2"], inputs["W3"], inputs["b3"], inputs["Wg"],
        inputs["bg"], inputs["Wv"], T,
    )
    res = run_bass_kernel_spmd(nc, in_maps, core_ids=list(range(N_CORES)))

    n_per_core = N // N_CORES
    out = np.empty((N, OUT, 3), dtype=np.float32)
    for i in range(N_CORES):
        o = res.results[i]["out"]
        out[i * n_per_core : (i + 1) * n_per_core] = o.transpose(2, 1, 0)[:n_per_core]
    return out



# revision 14
# speedup vs baseline: 2.5142x; 1.2721x over previous
"""Trainium2 Bass kernel for EquivariantPPFAttention (gnn_message_passing).

Contract: kernel(**inputs) takes FULL unsharded inputs (as produced by
reference.setup_inputs()) and returns the FULL [N, OUT, 3] float32 output.

Strategy (data-parallel over query points N across 8 NeuronCores):
  - shard q_pts / neighbor_indices across cores; replicate everything else.
  - one combined gather table comb[M, 512B]: s_feats row in bf16 (384B) +
    s_pts/normals in f32 (24B) + pad. Per query-tile of 128, dma_gather
    pulls all 128*32 neighbor rows (4 gathers of 1024 idxs - HW limit).
  - K-sum of the bf16 s_feats part on DVE (strided reduce, f32 accum);
    pts/normals extracted to a packed buffer for the PPF phase.
  - PPF angles via DVE arithmetic + ACT Sqrt/Arctan
    (atan2(r,y) = atan(r/y) + pi*[y<0] for r>=0); the four PPF features
    are written interleaved into one p4[:, t, ci, k] tensor.
  - tiny MLP on TensorE with rows on the free dim; two query-tiles packed
    per matmul via block-diagonal weights; mean-over-K folded into W3; the
    1/pi PPF normalization folded into W1; 1/K of the value path folded
    into Wv.
  - MLP input pack: per tile one PE transpose of p4 ([128 q, (ci k)] ->
    [(ci k), 128 q]) then ONE 4-row DMA (dst [4,4096] row-major ==
    src [128,128] partition-major traversal), alternating SP/ACT/Pool
    issue queues so no single sequencer serializes the pack; pf columns
    are k-major (col=k*128+q), so the K-mean reduce uses a strided view.
    h1/h2 run in f16 (weights cast on host); after the K-sum all f32.
    PSUM->SBUF staging copies ride the ACT engine to keep DVE free.
  - output DMAs issue from the Pool queue to keep the SP queue free for
    pack DMAs (in-order sequencers otherwise stall the next pair's pack).
"""

import math
import os
import numpy as np

N = 20000
M = 20000
K = 32
D = 64
HID = 64
OUT = 192
PPF_OUT = 64
N_CORES = 8
PI = math.pi

ES = 128          # f32 elems per comb row (512 B)
SFW = 96          # f32 slots holding the 192 bf16 s_feats values
PNO = 96          # f32 slot offset of pts/normals (6 floats)
NI = 1024         # idxs per dma_gather (HW-stable limit)
GPT = (128 * K) // NI   # gathers per query tile (4)
KPG = K // GPT    # k-blocks per gather (8)

_NC_CACHE = {}


def _build_nc(T, stage="full", loop=None):
    """Per-core Bass program for T query-tiles of 128.

    stage: debug bisection point - gather | ppf | mlp | full.
    loop: if set, repeat the whole body N times (for timing).
    """
    from contextlib import ExitStack, nullcontext
    from concourse import bacc, bass, mybir, tile

    assert T % 2 == 0
    NPAIR = T // 2
    NQ = 128 * T
    f32 = mybir.dt.float32
    bf16 = mybir.dt.bfloat16
    f16 = mybir.dt.float16
    i16 = mybir.dt.int16
    AF = mybir.ActivationFunctionType
    ALU = mybir.AluOpType

    nc = bacc.Bacc(
        "TRN2", target_bir_lowering=False, debug=False, num_swdge_queues=4
    )

    comb_in = nc.dram_tensor("comb", [M, ES], f32, kind="ExternalInput")
    qp_in = nc.dram_tensor("qp", [128, T, 3], f16, kind="ExternalInput")
    idx_in = nc.dram_tensor("idx16", [128, T, GPT, NI // 16], i16,
                            kind="ExternalInput")
    w1b_in = nc.dram_tensor("w1b", [8, 128], f16, kind="ExternalInput")
    b1b_in = nc.dram_tensor("b1b", [128, 1], f32, kind="ExternalInput")
    w2b_in = nc.dram_tensor("w2b", [128, 128], f16, kind="ExternalInput")
    b2b_in = nc.dram_tensor("b2b", [128, 1], f32, kind="ExternalInput")
    w3b_in = nc.dram_tensor("w3b", [128, 128], f32, kind="ExternalInput")
    b3b_in = nc.dram_tensor("b3b", [128, 1], f32, kind="ExternalInput")
    wgb_in = nc.dram_tensor("wgb", [128, 3, 128], f32, kind="ExternalInput")
    bgb_in = nc.dram_tensor("bgb", [128, 3], f32, kind="ExternalInput")
    wvb_in = nc.dram_tensor("wvb", [128, 3, 128], f32, kind="ExternalInput")
    ident_in = nc.dram_tensor("ident", [128, 128], f32, kind="ExternalInput")

    SIMPLEOUT = bool(int(os.environ.get("BENCH_SIMPLEOUT", "0")))
    if stage in ("full", "mlponly"):
        if SIMPLEOUT:
            out_dev = nc.dram_tensor(
                "out", [128, 9 * NQ], f32, kind="ExternalOutput"
            )
        else:
            out_dev = nc.dram_tensor(
                "out", [3, OUT, NQ], f32, kind="ExternalOutput"
            )
        dbg = None
    else:
        DBGW = {
            "gather": T * K * 8 + T * 192,
            "ppf": 4 * T * K,
            "mlp": 128 * K + 128 + 128 + 3 * 128,
            "gonly": T * 8,
            "gred": T * 8,
            "dbgdma": T * K * 8 + T * 192,
        }[stage]
        out_dev = None
        dbg = nc.dram_tensor("dbg", [128, DBGW], f32, kind="ExternalOutput")

    with tile.TileContext(nc) as tc, ExitStack() as ctx:
        const = ctx.enter_context(tc.tile_pool(name="const", bufs=1))
        gpool = ctx.enter_context(tc.tile_pool(name="gpool", bufs=2))
        gath = ctx.enter_context(tc.tile_pool(name="gath", bufs=1))
        planes = ctx.enter_context(tc.tile_pool(name="planes", bufs=1))
        temps = ctx.enter_context(tc.tile_pool(name="temps", bufs=2))
        mlpp = ctx.enter_context(tc.tile_pool(name="mlpp", bufs=1))
        small = ctx.enter_context(tc.tile_pool(name="small", bufs=2))
        psmlp = ctx.enter_context(tc.tile_pool(name="psmlp", bufs=3, space="PSUM"))
        pssm = ctx.enter_context(tc.tile_pool(name="pssm", bufs=3, space="PSUM"))
        pstp = ctx.enter_context(tc.tile_pool(name="pstp", bufs=2, space="PSUM"))

        def cload(name, dram, shape, dt=f32):
            t = const.tile(shape, dt, tag=name, name=name)
            if len(shape) > 3:
                dims = " ".join(f"d{i}" for i in range(len(shape) - 1))
                pat = f"p {dims} -> p ({dims})"
                nc.sync.dma_start(t[:].rearrange(pat), dram.ap().rearrange(pat))
            else:
                nc.sync.dma_start(t[:], dram.ap())
            return t

        qp_t = cload("qp", qp_in, [128, T, 3], f16)
        idx_t = cload("idx16", idx_in, [128, T, GPT, NI // 16], i16)
        w1b_t = cload("w1b", w1b_in, [8, 128], f16)
        b1b_t = cload("b1b", b1b_in, [128, 1])
        w2b_t = cload("w2b", w2b_in, [128, 128], f16)
        b2b_t = cload("b2b", b2b_in, [128, 1])
        w3b_t = cload("w3b", w3b_in, [128, 128])
        b3b_t = cload("b3b", b3b_in, [128, 1])
        wgb_t = cload("wgb", wgb_in, [128, 3, 128])
        bgb_t = cload("bgb", bgb_in, [128, 3])
        wvb_t = cload("wvb", wvb_in, [128, 3, 128])
        ident_t = cload("ident", ident_in, [128, 128])

        _loop_ctx = tc.For_i(0, loop, 1) if loop else nullcontext()
        with _loop_ctx:
            # ---- gather + per-tile K-reduce + pn extraction ----
            nbbuf = gath.tile([128, T, K, 8], f16, tag="nbbuf")
            sfsum = gath.tile([128, T, 192], f32, tag="sfsum")

            do_gather = stage not in ("dbgdma", "mlponly")
            do_reduce = stage not in ("gonly", "dbgdma", "mlponly")

            if stage in ("dbgdma", "mlponly"):
                nc.vector.memset(nbbuf[:].rearrange("p t k c -> p (t k c)"), 0.25)
                nc.vector.memset(sfsum[:].rearrange("p t c -> p (t c)"), 0.25)

            NORED = bool(int(os.environ.get("BENCH_NORED", "0")))
            if NORED and stage == "full":
                nc.vector.memset(nbbuf[:].rearrange("p t k c -> p (t k c)"), 0.25)
                nc.vector.memset(sfsum[:].rearrange("p t c -> p (t c)"), 0.25)

            gt_last = None

            def emit_gather_tile(t):
                nonlocal gt_last
                gt = gpool.tile([128, K, ES], f32, tag="gt", name="gt", bufs=3)
                gt_last = gt
                for g in range(GPT):
                    # queue g -> Q7 core pair g: the 4 gathers of one
                    # tile run concurrently on the 4 SWDGE core pairs.
                    nc.gpsimd.dma_gather(
                        out_ap=gt[:, g * KPG : (g + 1) * KPG, :],
                        in_ap=comb_in.ap(),
                        idxs_ap=idx_t[:, t, g, :],
                        num_idxs=NI,
                        num_idxs_reg=NI,
                        elem_size=ES,
                        queue_num=g,
                    )
                if do_reduce and not NORED:
                    # K-sum of the bf16 s_feats block via a contiguous
                    # tree-add (f16 intermediates, f32 final level)
                    gtb = gt[:].bitcast(bf16)          # [128, K, 256]
                    ga = gpool.tile([128, 16, 256], f16, tag="ga", bufs=2)
                    nc.vector.tensor_tensor(
                        ga[:, :, 0:192], gtb[:, 0:16, 0:192],
                        gtb[:, 16:32, 0:192], ALU.add,
                    )
                    for lv in (8, 4, 2):
                        nc.vector.tensor_tensor(
                            ga[:, 0:lv, 0:192], ga[:, 0:lv, 0:192],
                            ga[:, lv : 2 * lv, 0:192], ALU.add,
                        )
                    nc.vector.tensor_tensor(
                        sfsum[:, t, :], ga[:, 0, 0:192], ga[:, 1, 0:192],
                        ALU.add,
                    )
                    # pts/normals (f32) -> nbbuf[:, t, :, 0:6]
                    nc.scalar.activation(
                        nbbuf[:, t, :, 0:6], gt[:, :, PNO : PNO + 6], AF.Copy
                    )

            if do_gather and stage != "full":
                for t in range(T):
                    emit_gather_tile(t)
            if stage in ("gonly", "gred"):
                sb = gath.tile([128, T * 8], f32, tag="sdbg")
                nc.vector.tensor_copy(
                    sb[:],
                    nbbuf[:, 0, :, :].rearrange("p k c -> p (k c)")[:, : T * 8],
                )
                nc.sync.dma_start(dbg.ap()[:, : T * 8], sb[:])
            elif stage in ("gather", "dbgdma"):
                nc.sync.dma_start(
                    dbg.ap()[:, : T * K * 8],
                    nbbuf[:].rearrange("p t k c -> p (t k c)"),
                )
                nc.sync.dma_start(
                    dbg.ap()[:, T * K * 8 :],
                    sfsum[:].rearrange("p t c -> p (t c)"),
                )
            else:
                # ---- PPF + MLP, emitted in tile chunks so gather/PPF of
                # chunk c+1 overlaps the MLP pipeline of chunk c ----
                FW = T * K
                TT = nc.vector.tensor_tensor
                STT = nc.vector.scalar_tensor_tensor

                # interleaved PPF planes: p4[:, t, ci, k] (ci-major per tile)
                p4 = planes.tile([128, T, 4, K], f32, tag="p4", name="p4")

                def plane_view(ci):
                    # [128, FW] view of feature ci (strided; debug stages only)
                    return p4[:, :, ci, :].rearrange("p t k -> p (t k)")

                RW = 128 * K  # rows per query-tile (4096)
                HC = RW // 2
                NOPACK = bool(int(os.environ.get("BENCH_NOPACK", "0")))
                NOVAL = bool(int(os.environ.get("BENCH_NOVAL", "0")))
                NOGATE = bool(int(os.environ.get("BENCH_NOGATE", "0")))
                NOMLP12 = bool(int(os.environ.get("BENCH_NOMLP12", "0")))
                V1PACK = bool(int(os.environ.get("BENCH_V1PACK", "0")))
                PACKNODMA = bool(int(os.environ.get("BENCH_PACKNODMA", "0")))
                PACKNOTP = bool(int(os.environ.get("BENCH_PACKNOTP", "0")))

                if stage in ("full", "mlponly") and not SIMPLEOUT:
                    out_re = out_dev.ap().rearrange(
                        "c (jj p) q -> p c jj q", jj=3
                    )

                def emit_ppf(t0, tn):
                    with nc.allow_low_precision(reason="f16 PPF chain"):
                        _emit_ppf(t0, tn)

                def _emit_ppf(t0, tn):
                    # |a x b|^2 = |a|^2 |b|^2 - (a.b)^2 : no cross products.
                    nb = nbbuf[:, t0 : t0 + tn]
                    TS = nc.vector.tensor_scalar

                    def ttile(tag):
                        return temps.tile([128, tn * K], f16, tag=tag, name=tag)

                    def v3(t_):
                        return t_[:].rearrange("p (t k) -> p t k", k=K)

                    npv = [nb[:, :, :, c] for c in range(3)]
                    nnv = [nb[:, :, :, 3 + c] for c in range(3)]
                    qnv = [
                        nb[:, :, 0, 3 + c].to_broadcast([128, tn, K])
                        for c in range(3)
                    ]
                    qpv = [
                        qp_t[:, t0 : t0 + tn, c].to_broadcast([128, tn, K])
                        for c in range(3)
                    ]

                    vd = []
                    for c in range(3):
                        t_ = ttile(f"vd{c}")
                        TT(v3(t_), npv[c], qpv[c], ALU.subtract)
                        vd.append(t_)

                    def dot_views(av, bv, out_tag):
                        m0 = ttile("dm0")
                        TT(m0[:], av[0], bv[0], ALU.mult)
                        m1 = ttile("dm1")
                        TT(m1[:], av[1], bv[1], ALU.mult)
                        s = ttile(out_tag)
                        TT(s[:], m0[:], m1[:], ALU.add)
                        m2 = ttile("dm0")
                        TT(m2[:], av[2], bv[2], ALU.mult)
                        TT(s[:], s[:], m2[:], ALU.add)
                        return s

                    vdv = [v3(t_) for t_ in vd]

                    dd = dot_views(vdv, vdv, "dd")
                    y1 = dot_views(qnv, vdv, "y1")
                    y2 = dot_views(nnv, vdv, "y2")
                    y3 = dot_views(qnv, nnv, "y3")
                    nnsq = dot_views(nnv, nnv, "nnsq")

                    # |qn|^2 per query: [128, tn] (k=0 slice), ~free
                    qn_sq = temps.tile([128, tn], f16, tag="qnsq", name="qnsq")
                    qm = temps.tile([128, tn], f16, tag="qm")
                    TT(qn_sq[:], nb[:, :, 0, 3], nb[:, :, 0, 3], ALU.mult)
                    TT(qm[:], nb[:, :, 0, 4], nb[:, :, 0, 4], ALU.mult)
                    TT(qn_sq[:], qn_sq[:], qm[:], ALU.add)
                    TT(qm[:], nb[:, :, 0, 5], nb[:, :, 0, 5], ALU.mult)
                    TT(qn_sq[:], qn_sq[:], qm[:], ALU.add)
                    qnsq_b = qn_sq[:].to_broadcast([128, tn, K])

                    ddv = v3(dd)
                    nnsqv = v3(nnsq)

                    # msq_i = |a|^2 |b|^2 ; rs_i = max(msq_i - y_i^2, 0)
                    ys = [y1, y2, y3]
                    rss = []
                    for ci in range(3):
                        msq = ttile(f"msq{ci}")
                        if ci == 0:
                            TT(v3(msq), qnsq_b, ddv, ALU.mult)
                        elif ci == 1:
                            TT(v3(msq), nnsqv, ddv, ALU.mult)
                        else:
                            TT(v3(msq), qnsq_b, nnsqv, ALU.mult)
                        ysq = ttile("ysq")
                        TT(ysq[:], ys[ci][:], ys[ci][:], ALU.mult)
                        TT(msq[:], msq[:], ysq[:], ALU.subtract)
                        TS(msq[:], msq[:], 0.0, None, ALU.max)
                        rss.append(msq)

                    # ACT pass: Sqrt x4 (one table set), Reciprocal x3,
                    # then Arctan x3 (shares table with Relu/Sigmoid).
                    nc.scalar.activation(
                        p4[:, t0 : t0 + tn, 0, :], ddv, AF.Sqrt
                    )
                    rs_r = []
                    for ci in range(3):
                        r = ttile(f"r{ci}")
                        nc.scalar.activation(r[:], rss[ci][:], AF.Sqrt)
                        rs_r.append(r)
                    iys = []
                    for ci in range(3):
                        iy = ttile(f"iy{ci}")
                        nc.vector.reciprocal(iy[:], ys[ci][:])
                        iys.append(iy)

                    for ci in range(3):
                        tq = ttile("dm0")
                        TT(tq[:], rs_r[ci][:], iys[ci][:], ALU.mult)
                        at = ttile("dm1")
                        nc.scalar.activation(at[:], tq[:], AF.Arctan)
                        ind = ttile("ind")
                        TS(ind[:], ys[ci][:], 0.0, None, ALU.is_lt)
                        STT(
                            p4[:, t0 : t0 + tn, ci + 1, :],
                            v3(ind), PI, v3(at), ALU.mult, ALU.add,
                        )

                def emit_pair(j):
                    pf = mlpp.tile([8, RW], f16, tag="pf", bufs=2)
                    if NOPACK:
                        nc.vector.memset(pf[:], 0.25)
                    elif V1PACK:
                        for t2 in range(2):
                            t_abs = 2 * j + t2
                            for ci in range(4):
                                nc.sync.dma_start(
                                    pf[t2 * 4 + ci : t2 * 4 + ci + 1, :],
                                    p4[:, t_abs, ci, :],
                                )
                    else:
                        # pack via PE transpose: p4 tile slab [128 q, (ci k)]
                        # -> [(ci k), 128 q]; then 4 row-DMAs per tile with
                        # 512B descriptors, spread over 3 DMA-issue engines.
                        pts_s = mlpp.tile([128, 2, 128], f16, tag="pts", bufs=3)
                        if PACKNOTP:
                            nc.vector.memset(
                                pts_s[:].rearrange("p t q -> p (t q)"), 0.25
                            )
                        else:
                            for t2 in range(2):
                                t_abs = 2 * j + t2
                                tp = pstp.tile([128, 128], f32, tag="pstp")
                                nc.tensor.transpose(
                                    tp[:],
                                    p4[:, t_abs, :, :].rearrange(
                                        "p c k -> p (c k)"
                                    ),
                                    ident_t[:],
                                )
                                nc.scalar.activation(
                                    pts_s[:, t2, :], tp[:], AF.Copy
                                )
                        if PACKNODMA:
                            nc.vector.memset(pf[:], 0.25)
                        else:
                            # one DMA per tile: dst [4, 4096] row-major ==
                            # src [128, 128] partition-major traversal
                            # (dst col k*128+q <- src partition ci*32+k col q).
                            # Pool queues are reserved for the gathers now, so
                            # packs ride the two HWDGE rings (SP/ACT).
                            engs = [nc.sync, nc.scalar]
                            for t2 in range(2):
                                eng = engs[(j * 2 + t2) % 2]
                                eng.dma_start(
                                    pf[t2 * 4 : (t2 + 1) * 4, :],
                                    pts_s[:, t2, :],
                                )

                    # value-path transposes depend only on sfsum: run them
                    # on PE/ACT while the h1/h2 stream occupies the pipeline
                    if stage != "mlp" and not NOVAL:
                        av_e = sfsum[:, 2 * j : 2 * j + 2, :].rearrange(
                            "p t (d c) -> p c (t d)", c=3
                        )
                        aggs3 = small.tile([128, 3, 128], f32, tag="aggs3")
                        for c in range(3):
                            tpv = pssm.tile([128, 128], f32, tag="pssm")
                            nc.tensor.transpose(tpv[:], av_e[:, c, :], ident_t[:])
                            nc.scalar.activation(aggs3[:, c, :], tpv[:], AF.Copy)

                    ksum = small.tile([128, 128], f32, tag="ksum")
                    if NOMLP12:
                        nc.vector.memset(ksum[:], 0.25)
                    kparts = []
                    for hh in range(0 if NOMLP12 else 2):
                        h1s = mlpp.tile([128, HC], f16, tag="h1s", bufs=2)
                        for ch in range(HC // 512):
                            sl = slice(ch * 512, (ch + 1) * 512)
                            slg = slice(
                                hh * HC + ch * 512, hh * HC + (ch + 1) * 512
                            )
                            h1p = psmlp.tile([128, 512], f32, tag="psmlp")
                            nc.tensor.matmul(
                                h1p[:], w1b_t[:], pf[:, slg], start=True, stop=True
                            )
                            nc.scalar.activation(
                                h1s[:, sl], h1p[:], AF.Relu, bias=b1b_t[:]
                            )
                        h2s = mlpp.tile([128, HC], f16, tag="h2s", bufs=2)
                        for ch in range(HC // 512):
                            sl = slice(ch * 512, (ch + 1) * 512)
                            h2p = psmlp.tile([128, 512], f32, tag="psmlp")
                            nc.tensor.matmul(
                                h2p[:], w2b_t[:], h1s[:, sl], start=True, stop=True
                            )
                            if ch % 2 == 0:
                                nc.vector.tensor_scalar(
                                    h2s[:, sl], h2p[:], b2b_t[:], 0.0,
                                    ALU.add, ALU.max,
                                )
                            else:
                                nc.scalar.activation(
                                    h2s[:, sl], h2p[:], AF.Relu, bias=b2b_t[:]
                                )
                        # cols are k-major (col = k*128 + q): each hh holds
                        # k-block hh*16..hh*16+15 for all 128 queries.
                        # K-reduce via f16 tree adds (2x DVE mode) instead of
                        # tensor_reduce (1x mode).
                        h2v = h2s[:].rearrange("p (k q) -> p k q", q=128)
                        for lv in (8, 4, 2):
                            TT(
                                h2v[:, 0:lv, :], h2v[:, 0:lv, :],
                                h2v[:, lv : 2 * lv, :], ALU.add,
                            )
                        kp = small.tile([128, 128], f32, tag=f"kp{hh}")
                        TT(kp[:], h2v[:, 0, :], h2v[:, 1, :], ALU.add)
                        kparts.append(kp)
                    if not NOMLP12:
                        TT(ksum[:], kparts[0][:], kparts[1][:], ALU.add)

                    pmp = pssm.tile([128, 128], f32, tag="pssm")
                    nc.tensor.matmul(pmp[:], w3b_t[:], ksum[:], start=True, stop=True)
                    pms = small.tile([128, 128], f32, tag="pms")
                    nc.vector.tensor_scalar_add(pms[:], pmp[:], b3b_t[:])

                    if stage == "mlp" and j == 0:
                        nc.sync.dma_start(dbg.ap()[:, : RW // 2], h2s[:])
                        nc.sync.dma_start(dbg.ap()[:, RW : RW + 128], ksum[:])
                        nc.sync.dma_start(dbg.ap()[:, RW + 128 : RW + 256], pms[:])

                    gates = []
                    for jj in range(3):
                        gs = small.tile(
                            [128, 128], f32, tag=f"gate{jj}", name=f"gate{jj}"
                        )
                        if NOGATE:
                            nc.vector.memset(gs[:], 0.5)
                            gates.append(gs)
                            continue
                        gp = pssm.tile([128, 128], f32, tag="pssm")
                        nc.tensor.matmul(
                            gp[:], wgb_t[:, jj, :], pms[:], start=True, stop=True
                        )
                        nc.scalar.activation(
                            gs[:], gp[:], AF.Sigmoid, bias=bgb_t[:, jj : jj + 1]
                        )
                        gates.append(gs)
                        if stage == "mlp" and j == 0:
                            nc.sync.dma_start(
                                dbg.ap()[
                                    :,
                                    RW + 256 + jj * 128 : RW + 256 + (jj + 1) * 128,
                                ],
                                gs[:],
                            )
                    if stage == "mlp":
                        return

                    # value path: one transpose per component covers both
                    # tiles of the pair:
                    # in [128 q, (2 t x 64 d)] -> out [(2 t x 64 d), 128 q]
                    vstage = small.tile([128, 3, 3, 128], f32, tag="vstage")
                    if NOVAL:
                        nc.vector.memset(
                            vstage[:].rearrange("p a b c -> p (a b c)"), 0.25
                        )
                    for jj in range(0 if NOVAL else 3):
                        # one matmul covers all 3 spatial components (free
                        # dim 384) instead of 3x 128-wide; rides the 512-wide
                        # psmlp PSUM bank tiles.
                        vp3 = psmlp.tile([128, 512], f32, tag="psmlp")
                        nc.tensor.matmul(
                            vp3[:, 0:384],
                            wvb_t[:, jj, :],
                            aggs3[:].rearrange("p a b -> p (a b)"),
                            start=True, stop=True,
                        )
                        for c in range(3):
                            TT(
                                vstage[:, c, jj, :],
                                vp3[:, c * 128 : (c + 1) * 128],
                                gates[jj][:], ALU.mult,
                            )

                    for h in range(2):
                        q0 = (2 * j + h) * 128
                        if SIMPLEOUT:
                            nc.sync.dma_start(
                                out_dev.ap()[0:64, q0 * 9 : q0 * 9 + 9 * 128],
                                vstage[h * 64 : (h + 1) * 64, :, :, :].rearrange(
                                    "p c jj q -> p (c jj q)"
                                ),
                            )
                        else:
                            eng = nc.scalar if h == 0 else nc.sync
                            eng.dma_start(
                                out_re[:, :, :, q0 : q0 + 128].rearrange(
                                    "p c jj q -> p (c jj) q"
                                ),
                                vstage[h * 64 : (h + 1) * 64, :, :, :].rearrange(
                                    "p c jj q -> p (c jj) q"
                                ),
                            )

                if stage == "mlponly":
                    nc.vector.memset(
                        p4[:].rearrange("p t c k -> p (t c k)"), 0.25
                    )
                    for j in range(NPAIR):
                        emit_pair(j)
                elif stage == "ppf":
                    emit_ppf(0, T)
                    for ci in range(4):
                        nc.sync.dma_start(
                            dbg.ap()[:, ci * FW : (ci + 1) * FW], plane_view(ci)
                        )
                elif stage == "mlp":
                    emit_ppf(0, T)
                    emit_pair(0)
                else:
                    # interleave gather+reduce with PPF/MLP per chunk so
                    # every engine's (in-order) queue pipelines across
                    # chunks instead of draining all gathers first.
                    CT = int(os.environ.get("BENCH_CT", "4"))
                    assert T % CT == 0 and CT % 2 == 0
                    for t0 in range(0, T, CT):
                        if do_gather:
                            for t in range(t0, t0 + CT):
                                emit_gather_tile(t)
                        emit_ppf(t0, CT)
                        for j in range(t0 // 2, (t0 + CT) // 2):
                            emit_pair(j)

    nc.compile()
    return nc


def _f32_to_bf16_bits(x):
    """Round-to-nearest-even f32 -> bf16, returned as uint16 bits."""
    u = np.ascontiguousarray(x, dtype=np.float32).view(np.uint32)
    rounded = (u + 0x7FFF + ((u >> 16) & 1)) >> 16
    return rounded.astype(np.uint16)


def _host_prep(q_pts, s_pts, s_feats, neighbor_indices, normals,
               W1, b1, W2, b2, W3, b3, Wg, bg, Wv, T, n_total=N):
    NQ = 128 * T
    n_per_core = n_total // N_CORES
    f = np.float32

    comb = np.zeros((M, ES), dtype=f)
    cb = comb.view(np.uint16).reshape(M, ES * 2)
    cb[:, : 2 * SFW] = _f32_to_bf16_bits(s_feats.reshape(M, 192))
    comb[:, PNO : PNO + 3] = s_pts
    comb[:, PNO + 3 : PNO + 6] = normals

    W1T = W1.T.astype(f).copy()
    W1T[1:4] *= f(1.0 / PI)
    w1b = np.zeros((8, 128), dtype=f)
    w1b[0:4, 0:64] = W1T
    w1b[4:8, 64:128] = W1T
    b1b = np.concatenate([b1, b1]).astype(f)[:, None]

    def blockdiag2(A):
        n_, m_ = A.shape
        o = np.zeros((2 * n_, 2 * m_), dtype=f)
        o[:n_, :m_] = A
        o[n_:, m_:] = A
        return o

    w2b = blockdiag2(W2.T.astype(f))
    b2b = np.concatenate([b2, b2]).astype(f)[:, None]
    w3b = blockdiag2((W3.T / K).astype(f))
    b3b = np.concatenate([b3, b3]).astype(f)[:, None]

    WgT = Wg.T.astype(f)
    WvT = (Wv.T / K).astype(f)
    wgb = np.zeros((3, 128, 128), dtype=f)
    wvb = np.zeros((3, 128, 128), dtype=f)
    bgb = np.zeros((128, 3), dtype=f)
    for jj in range(3):
        wgb[jj] = blockdiag2(WgT[:, jj * 64 : (jj + 1) * 64])
        wvb[jj] = blockdiag2(WvT[:, jj * 64 : (jj + 1) * 64])
        bgb[:, jj] = np.concatenate([bg[jj * 64 : (jj + 1) * 64]] * 2)
    wgb_host = np.ascontiguousarray(wgb.transpose(1, 0, 2))
    wvb_host = np.ascontiguousarray(wvb.transpose(1, 0, 2))
    ident = np.eye(128, dtype=f)

    shared = dict(
        comb=comb, w1b=w1b.astype(np.float16), b1b=b1b,
        w2b=w2b.astype(np.float16), b2b=b2b, w3b=w3b, b3b=b3b,
        wgb=wgb_host, bgb=bgb, wvb=wvb_host, ident=ident,
    )

    in_maps = []
    for i in range(N_CORES):
        lo = i * n_per_core
        hi = lo + n_per_core
        qp_pad = np.zeros((NQ, 3), dtype=f)
        qp_pad[: hi - lo] = q_pts[lo:hi]
        idx_pad = np.zeros((NQ, K), dtype=np.int64)
        idx_pad[: hi - lo] = neighbor_indices[lo:hi]

        qp_host = np.ascontiguousarray(
            qp_pad.reshape(T, 128, 3).transpose(1, 0, 2)
        ).astype(np.float16)

        # idx16[p, t, g, s]: gather g of tile t covers logical rows
        # i' = (k - g*KPG)*128 + q, wrapped: w[l, s] = list[s*16 + l]
        idx16 = np.zeros((128, T, GPT, NI // 16), np.int16)
        for t in range(T):
            arr = idx_pad[t * 128 : (t + 1) * 128, :]      # [128 q, K]
            for g in range(GPT):
                lst = arr[:, g * KPG : (g + 1) * KPG].T.reshape(NI)
                idx16[:, t, g, :] = np.tile(
                    lst.reshape(NI // 16, 16).T.astype(np.int16), (8, 1)
                )

        m = dict(shared)
        m.update(qp=qp_host, idx16=idx16)
        in_maps.append(m)
    return in_maps


def kernel(**inputs):
    from concourse.bass_utils import run_bass_kernel_spmd

    T = 20
    inputs = {k: np.asarray(v) for k, v in inputs.items()}
    idx = inputs["neighbor_indices"].astype(np.int64)

    if T not in _NC_CACHE:
        _NC_CACHE[T] = _build_nc(T)
    nc = _NC_CACHE[T]

    in_maps = _host_prep(
        inputs["q_pts"], inputs["s_pts"], inputs["s_feats"], idx,
        inputs["normals"], inputs["W1"], inputs["b1"], inputs["W2"],
        inputs["b2"], inputs["W3"], inputs["b3"], inputs["Wg"],
        inputs["bg"], inputs["Wv"], T,
    )
    res = run_bass_kernel_spmd(nc, in_maps, core_ids=list(range(N_CORES)))

    n_per_core = N // N_CORES
    out = np.empty((N, OUT, 3), dtype=np.float32)
    for i in range(N_CORES):
        o = res.results[i]["out"]
        out[i * n_per_core : (i + 1) * n_per_core] = o.transpose(2, 1, 0)[:n_per_core]
    return out

